# revision 82
# baseline (speedup 1.0000x reference)
"""Trainium2 Bass kernel for nn_MultiHeadTransformerPosEmb.

Output is `y[:, 0, :] @ wu.T` — only the CLS row feeds the unembedding, so per
batch only ONE attention query row is needed; the [B,H,S,S] score tensor never
materializes.

Distribution over 8 NeuronCores:
 - batch-parallel attention: core b computes z_b = wo @ (attn row 0) + 2*y0
 - 3-round XOR-hypercube all-gather of z (descriptor-gen hoisted to t~0,
   data-gated triggers) instead of 7 serial broadcasts
 - vocab-parallel unembed, TRANSPOSED: vocab chunks on PE partitions, batch on
   the free dim (ap_size 8/matmul instead of 512 — 16x less PE time)

Precision plan (rel-err budget 2e-2): the output is dominated by the residual
2*y0 = 2*(emb[2] + pe[0]).  The hidden dim is host-permuted to [odd dims,
even dims] so pe[0] becomes [1...1, 0...0]; wu columns for the odd (large-z)
half stay bf16 while the even (small-z) half drops to fp8.  pe and all
attention weights (wq/wk/wv/wo) are fp8 — they only touch the small attention
contribution.  Host-side prep is limited to index/layout/dtype transforms.
"""
import math
from contextlib import ExitStack

import numpy as np
import ml_dtypes

import concourse.bass as bass
import concourse.tile as tile
import concourse.mybir as mybir
from concourse import bacc, bass_utils
from concourse.tile_rust import add_dep_helper

F32 = mybir.dt.float32
BF16 = mybir.dt.bfloat16
FP8 = mybir.dt.float8e4
I16 = mybir.dt.int16

NCORES = 8
B = 8
SEQ = 2048          # S + 1 (CLS prepended)
H = 256             # hidden
NH = 8              # heads
HD = 32             # head dim
V = 32001
VPAD = 32768
VSLICE = VPAD // NCORES   # 4096
NCH = VSLICE // 128       # 32 vocab chunks per core
NT = SEQ // 128           # 16 position tiles
HSEQ = SEQ // 2
SCALE = 1.0 / math.sqrt(HD)

# hidden-dim permutation: odd dims first (pe row0 == 1), even dims second
# (pe row0 == 0).  Applied on the host to every tensor's hidden axis.
PERM = np.concatenate([np.arange(1, H, 2), np.arange(0, H, 2)])

# fp8 pack layout (columns of pack8 [128, 2, 3080])
P_PE1 = 0           # positions 0:1024
P_WQ = 1024
P_WK = 1280
P_PE2 = 1536        # positions 1024:2048
P_WV = 2560
P_WO = 2816
P_MC = 3072         # maskc [.., 8]
P8 = 3080

_CACHE = {}


def _build(attach_exchange_wait=True):
    nc = bacc.Bacc("TRN2", target_bir_lowering=False, debug=False,
                   num_devices=NCORES)

    emb = nc.dram_tensor("emb", [V, H], BF16, kind="ExternalInput")
    idxs = nc.dram_tensor("idxs", [128, SEQ // 16], I16, kind="ExternalInput")
    pack8 = nc.dram_tensor("pack8", [128, 2, P8], FP8, kind="ExternalInput")
    packm = nc.dram_tensor("packm", [128, 3], F32, kind="ExternalInput")
    maskt = nc.dram_tensor("maskt", [NH, H], F32, kind="ExternalInput")
    wu_hi = nc.dram_tensor("wu_hi", [128, VSLICE], BF16, kind="ExternalInput")
    wu_lo = nc.dram_tensor("wu_lo", [128, VSLICE], FP8, kind="ExternalInput")
    out = nc.dram_tensor("out", [128, NCH, B], BF16, kind="ExternalOutput")
    rankmap = nc.dram_tensor("rankmap", [1, B], BF16, kind="ExternalOutput")

    with tile.TileContext(nc) as tc, ExitStack() as ctx:
        cp = ctx.enter_context(tc.tile_pool(name="const", bufs=1))
        wp = ctx.enter_context(tc.tile_pool(name="work", bufs=2))
        bigp = ctx.enter_context(tc.tile_pool(name="big", bufs=1))
        rxp = ctx.enter_context(tc.tile_pool(name="rx", bufs=1))
        # PSUM: 8 banks total = vp2 + sp2 + qp1 + op(2 tags) + up1
        vp = ctx.enter_context(tc.tile_pool(name="vps", bufs=3, space="PSUM"))
        sp = ctx.enter_context(tc.tile_pool(name="sps", bufs=1, space="PSUM"))
        qp = ctx.enter_context(tc.tile_pool(name="qps", bufs=2, space="PSUM"))
        op = ctx.enter_context(tc.tile_pool(name="ops", bufs=1, space="PSUM"))
        up = ctx.enter_context(tc.tile_pool(name="ups", bufs=1, space="PSUM"))

        # ---- DMA issue order (SP queue / shared HWDGE+DMA-engine FIFO) ------
        idx_sb = cp.tile([128, SEQ // 16], I16)
        nc.sync.dma_start(idx_sb[:], idxs[:])
        p8 = cp.tile([128, 2, P8], FP8)
        nc.sync.dma_start(p8[:, :, 0:P_PE2], pack8[:, :, 0:P_PE2])
        nc.sync.dma_start(p8[:, :, P_PE2:], pack8[:, :, P_PE2:])
        pm = cp.tile([128, 3], F32)
        nc.sync.dma_start(pm[:], packm[:])
        mt = cp.tile([NH, H], F32)
        nc.sync.dma_start(mt[:], maskt[:])
        wu_hi_sb = cp.tile([128, VSLICE], BF16)
        wu_lo_sb = cp.tile([128, VSLICE], FP8)
        HV = VSLICE // 2

        # views into the packs
        peA = p8[:, :, P_PE1:P_PE1 + 1024]     # positions 0:1024
        peB = p8[:, :, P_PE2:P_PE2 + 1024]     # positions 1024:2048
        wq_sb = p8[:, :, P_WQ:P_WQ + H]
        wk_sb = p8[:, :, P_WK:P_WK + H]
        wv_sb = p8[:, :, P_WV:P_WV + H]
        wo_sb = p8[:, :, P_WO:P_WO + H]
        maskc_sb = p8[:, :, P_MC:P_MC + NH]

        # ---- embedding gather (transposed, bf16) → yT_emb -------------------
        yT_emb = [bigp.tile([128, 2, HSEQ], BF16, tag=f"yemb{h}",
                            name=f"yemb{h}") for h in range(2)]
        gather_i = []
        for h in range(2):
            gather_i.append(nc.gpsimd.dma_gather(
                out_ap=yT_emb[h][:], in_ap=emb[:],
                idxs_ap=idx_sb[:, bass.ts(h, HSEQ // 16)],
                num_idxs=HSEQ, num_idxs_reg=HSEQ, elem_size=H, transpose=True,
                single_packet=False,
            ))

        # ---- exchange descriptor-gen, hoisted to t~0 ------------------------
        # zcol cols: [z_half0, z_half1, rank, pad].  zt slot k will hold the
        # payload of core (me XOR k); host un-permutes rows via the rank col.
        # zcol is written LATE (before the triggers) so the desc-gens are
        # ungated — descriptors only encode addresses, data is read at trigger.
        zt = rxp.tile([128, B, 4], BF16)
        zcol = zt[:, 0, :]
        rsem1 = nc.alloc_semaphore("rx1")
        rsem2 = nc.alloc_semaphore("rx2")
        rsem3 = nc.alloc_semaphore("rx3")
        lsem = nc.alloc_semaphore("rdma_local")
        rd1 = [None] * NCORES
        rd1[1] = (0, 1)
        d1 = nc.gpsimd.remote_dma_broadcast(
            out_ap=zt[:, 1, :], in_ap=zcol[:],
            remote_sem=rsem1, local_sem=lsem, rdests=rd1)
        rd2 = [None] * NCORES
        rd2[2] = (0, 2)
        d2 = nc.gpsimd.remote_dma_broadcast(
            out_ap=zt[:, 2:4, :], in_ap=zt[:, 0:2, :],
            remote_sem=rsem2, local_sem=lsem, rdests=rd2)
        rd3 = [None] * NCORES
        rd3[4] = (0, 4)
        d3 = nc.gpsimd.remote_dma_broadcast(
            out_ap=zt[:, 4:8, :], in_ap=zt[:, 0:4, :],
            remote_sem=rsem3, local_sem=lsem, rdests=rd3)
        for d in (d1, d2, d3):
            # ordering-only: keep the Pool SEQ free for the gather desc-gen
            # dispatches first
            add_dep_helper(d.ins, gather_i[1].ins, sync=False,
                           reason="rdma desc-gen after gather desc dispatch")

        ones_sb = cp.tile([128, 1], BF16)
        nc.vector.memset(ones_sb[:], 1.0)

        # ---- y0 = emb[2] + pe[0]; pe[0] is [1...,0...] after the perm -------
        y0f = wp.tile([128, 2, 1], F32, tag="y0f")
        nc.vector.tensor_scalar_add(y0f[:, 0, :], pm[:, 0:1], 1.0)
        nc.vector.tensor_copy(out=y0f[:, 1, :], in_=pm[:, 1:2])
        y0b = wp.tile([128, 2, 1], BF16, tag="y0b")
        nc.vector.tensor_copy(out=y0b[:], in_=y0f[:])

        # ---- q0 (scaled), block-diag columns bd, fused qk -------------------
        bd_sb = wp.tile([128, 2, NH], BF16, tag="bd")
        for m in range(2):
            qps = qp.tile([128, NH], F32, tag="small")
            for c in range(2):
                nc.tensor.matmul(qps[:, 0:1], lhsT=wq_sb[:, c, bass.ts(m, 128)],
                                 rhs=y0b[:, c, :], start=(c == 0), stop=(c == 1))
            q0c = wp.tile([128, 1], BF16, tag="q0c")
            nc.scalar.mul(q0c[:], qps[:, 0:1], SCALE)
            nc.vector.tensor_tensor(out=bd_sb[:, m, :],
                                    in0=q0c[:].to_broadcast([128, NH]),
                                    in1=maskc_sb[:, m, :],
                                    op=mybir.AluOpType.mult)
        qk_sb = wp.tile([128, 2, NH], BF16, tag="qk")
        for m in range(2):
            qkps = qp.tile([128, NH], F32, tag="small")
            for c in range(2):
                nc.tensor.matmul(qkps[:], lhsT=wk_sb[:, c, bass.ts(m, 128)],
                                 rhs=bd_sb[:, c, :], start=(c == 0), stop=(c == 1))
            nc.vector.tensor_copy(out=qk_sb[:, m, :], in_=qkps[:])

        # ---- wu loads, explicitly ordered after the first gather half so
        # they enter the serial DMA FIFO after both gather transfers ----------
        wu_dmas = [
            nc.sync.dma_start(wu_hi_sb[:, 0:HV], wu_hi[:, 0:HV]),
            nc.sync.dma_start(wu_lo_sb[:, 0:HV], wu_lo[:, 0:HV]),
            nc.sync.dma_start(wu_hi_sb[:, HV:], wu_hi[:, HV:]),
            nc.sync.dma_start(wu_lo_sb[:, HV:], wu_lo[:, HV:]),
        ]
        for wdma in wu_dmas:
            add_dep_helper(wdma.ins, gather_i[0].ins,
                           reason="wu enters DMA FIFO after gathers")

        # fp8 -> bf16 upconvert of pe on ACT (fast there), so the yT adds hit
        # the DVE 2-byte fast path
        pe_bf = bigp.tile([128, 2, SEQ], BF16, tag="pebf")
        Q = HSEQ // 2
        for c in range(2):
            for q in range(2):
                nc.scalar.copy(pe_bf[:, c, q * Q:(q + 1) * Q],
                               peA[:, c, bass.ts(q, Q)])
        for c in range(2):
            for q in range(2):
                nc.scalar.copy(pe_bf[:, c, HSEQ + q * Q:HSEQ + (q + 1) * Q],
                               peB[:, c, bass.ts(q, Q)])

        def pe_tile(t, c):
            h, tt = divmod(t, 8)
            src = peA if h == 0 else peB
            return src[:, c, bass.ts(tt, 128)]

        def emb_tile(t, c):
            h, tt = divmod(t, 8)
            return yT_emb[h][:, c, bass.ts(tt, 128)]

        # ---- scoresT + exp, split y = emb + pe into two matmul partials so
        # the exp path does not wait on the elementwise add ----------------
        aT_all = bigp.tile([128, NT, NH], BF16)

        def scores_half(h):
            for g in range(2):
                sps = sp.tile([128, 4, NH], F32)
                for j in range(4):
                    t = 8 * h + 4 * g + j
                    for c in range(2):
                        nc.tensor.matmul(sps[:, j, :], lhsT=pe_tile(t, c),
                                         rhs=qk_sb[:, c, :],
                                         start=(c == 0), stop=False)
                    for c in range(2):
                        nc.tensor.matmul(sps[:, j, :], lhsT=emb_tile(t, c),
                                         rhs=qk_sb[:, c, :],
                                         start=False, stop=(c == 1))
                nc.scalar.activation(out=aT_all[:, bass.ts(2 * h + g, 4), :],
                                     in_=sps[:],
                                     func=mybir.ActivationFunctionType.Exp)
                for j in range(4):
                    t = 8 * h + 4 * g + j
                    nc.tensor.matmul(den, lhsT=aT_all[:, t, :], rhs=ones_sb[:],
                                     start=(t == 0), stop=(t == NT - 1))

        # ---- yT = emb + pe (adds split across DVE and Pool), v = yT @ Wv ----
        yT = bigp.tile([128, 2, SEQ], BF16)
        v_all = bigp.tile([128, NT, H], BF16)

        def add_half(h):
            # all-bf16 operands -> DVE 2x/4x fast path
            for c in range(2):
                nc.vector.tensor_tensor(
                    out=yT[:, c, bass.ts(h, HSEQ)],
                    in0=yT_emb[h][:, c, :],
                    in1=pe_bf[:, c, bass.ts(h, HSEQ)], op=mybir.AluOpType.add)

        def v_half(h):
            for pair in range(4):
                vps = vp.tile([128, 2, H], F32)
                for j in range(2):
                    t = 8 * h + 2 * pair + j
                    for c in range(2):
                        nc.tensor.matmul(vps[:, j, :],
                                         lhsT=yT[:, c, bass.ts(t, 128)],
                                         rhs=wv_sb[:, c, :],
                                         start=(c == 0), stop=(c == 1))
                gp = 4 * h + pair
                dst = v_all[:, bass.ts(gp, 2), :]
                if gp % 2 == 1:
                    nc.scalar.copy(dst, vps[:])
                else:
                    nc.vector.tensor_copy(out=dst, in_=vps[:])

        # ---- transposed o-acc: o0T[hd, head] = sum_pos v.T @ aT -------------
        # oacc halves and the softmax denominator share one PSUM bank
        oaccd = op.tile([128, 3, NH], F32, tag="oacc")
        oacc = oaccd[:, 0:2, :]
        den = oaccd[0:NH, 2, 0:1]

        def oacc_half(h):
            for tt in range(8):
                t = 8 * h + tt
                st, sp_ = (t == 0), (t == NT - 1)
                nc.tensor.matmul(oacc[:, 0, :], lhsT=v_all[:, t, 0:128],
                                 rhs=aT_all[:, t, :], start=st, stop=sp_)
                nc.tensor.matmul(oacc[:, 1, :], lhsT=v_all[:, t, 128:256],
                                 rhs=aT_all[:, t, :], start=st, stop=sp_)

        # PE keep-warm dummies: the p-state model only reaches full clock
        # after ~3us of continuous execution, and any engine gap resets it.
        # Two throwaway matmuls bridge the data-arrival gaps before v1.
        def dummy_mm(ncols, c):
            dps = qp.tile([128, NH], F32, tag="small")
            nc.tensor.matmul(dps[:, 0:1], lhsT=wq_sb[:, c, 0:128],
                             rhs=p8[:, c, 0:ncols], start=True, stop=True,
                             skip_group_check=True)

        scores_half(0)
        scores_half(1)
        add_half(0)
        add_half(1)
        v_half(0)
        v_half(1)
        oacc_half(0)
        oacc_half(1)

        # softmax denominator chain: high priority so the scheduler slots the
        # recip/rexp work as soon as the den accumulation stops, well before
        # the oacc accumulation finishes
        with tc.high_priority():
            den_sb = wp.tile([NH, 1], F32, tag="den_sb")
            nc.vector.tensor_copy(out=den_sb[:], in_=den[:])
            recip = wp.tile([NH, 1], F32, tag="recip")
            nc.vector.reciprocal(recip[:], den_sb[:])
            rexp_sb = wp.tile([128, 2, 1], F32, tag="rexp")
            for c in range(2):
                rexps = qp.tile([128, NH], F32, tag="small")
                nc.tensor.matmul(rexps[:, 0:1], lhsT=mt[:, bass.ts(c, 128)],
                                 rhs=recip[:], start=True, stop=True)
                nc.vector.tensor_copy(out=rexp_sb[:, c, :], in_=rexps[:, 0:1])
        om = wp.tile([128, 2, NH], F32, tag="om")
        nc.vector.tensor_tensor(out=om[:], in0=oacc[:], in1=maskc_sb[:],
                                op=mybir.AluOpType.mult)
        osel = wp.tile([128, 2, 1], F32, tag="osel")
        nc.vector.tensor_reduce(out=osel[:], in_=om[:],
                                axis=mybir.AxisListType.X,
                                op=mybir.AluOpType.add)
        oc = wp.tile([128, 2, 1], BF16, tag="oc")
        nc.vector.tensor_tensor(out=oc[:], in0=osel[:], in1=rexp_sb[:],
                                op=mybir.AluOpType.mult)
        zw = [nc.vector.memset(zcol[:, 2:4], 0.0),
              nc.vector.tensor_copy(out=zcol[0:1, 2:3], in_=pm[0:1, 2:3])]
        for m in range(2):
            zps = qp.tile([128, NH], F32, tag="small")
            for c in range(2):
                nc.tensor.matmul(zps[:, 0:1], lhsT=wo_sb[:, c, bass.ts(m, 128)],
                                 rhs=oc[:, c, :], start=(c == 0), stop=(c == 1))
            zw.append(nc.vector.scalar_tensor_tensor(
                out=zcol[:, m:m + 1], in0=y0f[:, m, :], scalar=2.0,
                in1=zps[:, 0:1], op0=mybir.AluOpType.mult,
                op1=mybir.AluOpType.add))

        # ---- fire the exchange; explicit deps keep the trigger chain after
        # the zcol/zt data writes (the scheduler would otherwise float them) --
        t1 = nc.gpsimd.trigger_dma(count=1)
        for w in zw:
            add_dep_helper(t1.ins, w.ins, reason="fire r1 after z writes")
        add_dep_helper(t1.ins, d1.ins, reason="r1 descs before trigger")
        t2 = nc.gpsimd.trigger_dma(count=1)
        add_dep_helper(t2.ins, t1.ins, reason="round order")
        add_dep_helper(t2.ins, d2.ins, reason="r2 descs before trigger")
        t3 = nc.gpsimd.trigger_dma(count=1)
        add_dep_helper(t3.ins, t2.ins, reason="round order")
        add_dep_helper(t3.ins, d3.ins, reason="r3 descs before trigger")
        b3 = nc.vector.tensor_copy(out=zt[:, 1:8, :], in_=zt[:, 1:8, :])
        add_dep_helper(b3.ins, t3.ins, reason="slots land after rounds fire")
        nc.sync.dma_start(rankmap[:], zt[0:1, :, 2])

        # ---- transposed unembed: outT[j, b] = wu[j, :].T-chunks @ z ---------
        ups = up.tile([128, NCH, B], F32)
        osb = wp.tile([128, NCH, B], BF16, tag="osb")
        for half in range(2):
            for ch in range(NCH // 2):
                chg = half * (NCH // 2) + ch
                nc.tensor.matmul(ups[:, chg, :],
                                 lhsT=wu_hi_sb[:, bass.ts(chg, 128)],
                                 rhs=zt[:, :, 0], start=True, stop=False)
                nc.tensor.matmul(ups[:, chg, :],
                                 lhsT=wu_lo_sb[:, bass.ts(chg, 128)],
                                 rhs=zt[:, :, 1], start=False, stop=True)
            dsl = bass.ts(half, NCH // 2)
            if half == 0:
                nc.vector.tensor_copy(out=osb[:, dsl, :], in_=ups[:, dsl, :])
            else:
                nc.scalar.copy(osb[:, dsl, :], ups[:, dsl, :])
            nc.sync.dma_start(out[:, dsl, :], osb[:, dsl, :])

    if attach_exchange_wait:
        # cross-core arrival gates; attached post-scheduling (the Tile
        # scheduler's single-core sim cannot satisfy them)
        t2.wait_op(rsem1, 2, "sem-ge", check=False)
        t3.wait_op(rsem2, 2, "sem-ge", check=False)
        b3.wait_op(rsem1, 2, "sem-ge", check=False)
        b3.wait_op(rsem2, 2, "sem-ge", check=False)
        b3.wait_op(rsem3, 2, "sem-ge", check=False)
    nc.finalize()
    return nc


def _pos_encoding_np():
    pos = np.arange(SEQ, dtype=np.float32)[:, None]
    div = np.exp(np.arange(0, H, 2, dtype=np.float32)
                 * np.float32(-(math.log(10000.0) / H)))
    ang = pos * div[None, :]
    pe = np.zeros((SEQ, H), dtype=np.float32)
    pe[:, 0::2] = np.sin(ang)
    pe[:, 1::2] = np.cos(ang)
    return pe


def _part_chunk(a2d):
    """[256, N] -> [128, 2, N] with [p, c, :] = a2d[c*128 + p]."""
    n = a2d.shape[1]
    return np.ascontiguousarray(a2d.reshape(2, 128, n).transpose(1, 0, 2))


def prepare_in_maps(x, emb_w, wq, wk, wv, wo, wu):
    x = np.asarray(x)
    emb_w = np.asarray(emb_w, dtype=np.float32)
    wq = np.asarray(wq, dtype=np.float32)
    wk = np.asarray(wk, dtype=np.float32)
    wv = np.asarray(wv, dtype=np.float32)
    wo = np.asarray(wo, dtype=np.float32)
    wu = np.asarray(wu, dtype=np.float32)

    tok = np.concatenate(
        [np.full((B, 1), 2, dtype=np.int64), x], axis=1).astype(np.int16)

    emb_host = emb_w[:, PERM].astype(ml_dtypes.bfloat16)

    fp8 = ml_dtypes.float8_e4m3fn
    pack8_host = np.zeros((128, 2, P8), dtype=fp8)
    peT = _part_chunk(_pos_encoding_np()[:, PERM].T)               # [128,2,SEQ]
    pack8_host[:, :, P_PE1:P_PE1 + 1024] = peT[:, :, 0:1024].astype(fp8)
    pack8_host[:, :, P_PE2:P_PE2 + 1024] = peT[:, :, 1024:2048].astype(fp8)
    pack8_host[:, :, P_WQ:P_WQ + H] = _part_chunk(
        wq.reshape(H, H).T[PERM]).astype(fp8)
    pack8_host[:, :, P_WK:P_WK + H] = _part_chunk(
        wk.reshape(H, H)[:, PERM]).astype(fp8)
    pack8_host[:, :, P_WV:P_WV + H] = _part_chunk(
        wv.reshape(H, H).T[PERM]).astype(fp8)
    pack8_host[:, :, P_WO:P_WO + H] = _part_chunk(
        wo.T[:, PERM]).astype(fp8)
    hd_idx = np.arange(H) // HD
    maskc = np.ascontiguousarray(
        (hd_idx.reshape(2, 128)[:, :, None] == np.arange(NH)[None, None, :])
        .astype(fp8).transpose(1, 0, 2))                           # [128,2,8]
    pack8_host[:, :, P_MC:P_MC + NH] = maskc

    maskt_host = (hd_idx[None, :] == np.arange(NH)[:, None]).astype(np.float32)
    e2c = _part_chunk(emb_w[2][PERM].reshape(H, 1))                # [128,2,1]

    wu_pad = np.zeros((VPAD, H), dtype=np.float32)
    wu_pad[:V] = wu
    wu_perm = wu_pad[:, PERM]

    in_maps = []
    for core in range(NCORES):
        tb = tok[core]
        idx_t = np.tile(np.ascontiguousarray(tb.reshape(SEQ // 16, 16).T),
                        (8, 1))                                    # [128, 128]
        packm_host = np.zeros((128, 3), dtype=np.float32)
        packm_host[:, 0:2] = e2c[:, :, 0]
        packm_host[0, 2] = float(core)
        sl = wu_perm[VSLICE * core: VSLICE * (core + 1)]           # [4096, 256]
        in_maps.append({
            "emb": emb_host, "idxs": idx_t,
            "pack8": pack8_host, "packm": packm_host, "maskt": maskt_host,
            "wu_hi": np.ascontiguousarray(sl[:, 0:128].T).astype(
                ml_dtypes.bfloat16),
            "wu_lo": np.ascontiguousarray(sl[:, 128:256].T).astype(fp8),
        })
    return in_maps


def get_nc():
    if "nc" not in _CACHE:
        _CACHE["nc"] = _build()
    return _CACHE["nc"]


def get_timing_nc():
    """Variant without the cross-core sem waits, for single-core TimelineSim.

    Slightly optimistic: it omits the waits for peers' payload arrival
    (~1-2us of skew on real hardware).
    """
    return _build(attach_exchange_wait=False)


def assemble(results):
    full = np.zeros((B, VPAD), dtype=np.float32)
    for core in range(NCORES):
        ranks = np.asarray(results[core]["rankmap"]).astype(np.int32).ravel()
        blk = np.asarray(results[core]["out"]).astype(np.float32)
        blk = blk.transpose(2, 1, 0).reshape(B, VSLICE)            # [slot, j]
        for slot in range(B):
            full[ranks[slot], VSLICE * core: VSLICE * (core + 1)] = blk[slot]
    return np.ascontiguousarray(full[:, :V])


def kernel(x, emb_w, wq, wk, wv, wo, wu):
    nc = get_nc()
    in_maps = prepare_in_maps(x, emb_w, wq, wk, wv, wo, wu)
    res = bass_utils.run_bass_kernel_spmd(
        nc, in_maps, core_ids=list(range(NCORES)))
    return assemble(res.results)
